# revision 2
# baseline (speedup 1.0000x reference)
"""Trainium2 Bass kernel v2.2 for NeuralClusteringAttention.

Tokens are sorted by cluster on host -> attention is exactly block-diagonal
per cluster slot: keys for a slot's queries are a column window of K and a
partition block of V, so no mask rows are needed.

Reference softmax semantics (masked scores are 0, not -inf) are reproduced
with a zero-X "denominator token" per slot whose V ones-columns carry weight
(T - n_c - pad_cnt): pad keys contribute exp(0)*1 to the mass and 0 to
values; the reduced denominator weight cancels them exactly.

Everything runs in float32r: self-loading weights (no per-matmul Ldweights
on PE.SEQ) at 1 cycle/col for >=256-wide moving dims. Attention matmuls pad
their moving dim to 256 (T2 is extended so reads stay in bounds); the exp
only covers the real slot width, and each p1 tile's pad columns are
zero-memset on the idle Pool engine so the padded PV reads stay finite.
Heads are ordered [0,2,1,3] in the score/PV psum so each psum bank's
accumulation group keeps a uniform PE tile_position.

Emission is software-pipelined: score(s+1) issues on PE before PV(s), so
exp(s) latency on ACT hides behind real PE work, and each engine queue's
in-order stream matches data-readiness order.

Sharding: 8 cores = (4 batches) x (2 head-halves of 4 heads each). Host
sums the two fp16 partials per batch and adds bout.
"""

import numpy as np

import concourse.bacc as bacc
import concourse.bass as bass
import concourse.mybir as mybir
import concourse.tile as tile
from concourse.bass_utils import run_bass_kernel_spmd

B, T, D, C, H = 4, 1024, 512, 8, 8
HD = D // H          # 64
LH = 4               # local heads per core
F32 = mybir.dt.float32
F16 = mybir.dt.float16
MMDT = mybir.dt.float32r
HO = (0, 2, 1, 3)    # head slot order: uniform tile_position per psum bank
WPAD = 256           # attention moving-dim pad (fp32r full rate at >=256)


def make_schedule(assign_all):
    """Canonical per-slot layout shared by all batches (single SPMD program)."""
    counts = np.stack([np.bincount(a, minlength=C) for a in assign_all])
    sizes = -np.sort(-counts, axis=1)                      # [B, C] desc
    Ws = ((sizes.max(axis=0) + 1 + 3) // 4) * 4            # canonical widths
    As = np.zeros(C, np.int64)
    As[1:] = np.cumsum(Ws)[:-1]
    used = int(As[-1] + Ws[-1])
    T2 = ((used + 15) // 16) * 16
    return T2, [int(w) for w in Ws], [int(a) for a in As]


def _chunks(T2):
    """Column chunks for qk/out/xt: <=512 (one psum bank) and >=256 wide
    (fp32r full rate), last two balanced."""
    ch = []
    o = 0
    while o < T2:
        rem = T2 - o
        w = rem if rem <= 512 else (512 if rem >= 768 else (rem + 1) // 2 // 8 * 8)
        ch.append((o, o + w))
        o += w
    return ch


def _kernel_body(tc, T2, Ws, As):
    nc = tc.nc
    QCH = _chunks(T2)
    assert max(Ws) <= WPAD
    # key blocks per slot
    kunits = []
    for s, (a, w) in enumerate(zip(As, Ws)):
        kunits.append((s, 0, min(w, 128)))
        if w > 128:
            kunits.append((s, 128, w))
    # v block index: slot-leading blocks 0..7 so the denominator patch is one
    # regular-stride DMA; overflow blocks follow
    vblk = {}
    novf = 0
    for s, klo, khi in kunits:
        if klo == 0:
            vblk[(s, 0)] = s
        else:
            vblk[(s, klo)] = C + novf
            novf += 1
    NVB = C + novf

    # slot groups by the qk chunk their padded score window needs
    ends = [min(a + WPAD, T2) for a in As]  # shifted windows end here
    sgrp = [[] for _ in QCH]
    for s in range(C):
        ci = next(i for i, (c0, c1) in enumerate(QCH) if ends[s] <= c1)
        sgrp[ci].append(s)

    xt_d = nc.dram_tensor("xt", [D, T2], F32, kind="ExternalInput").ap()
    wqk_d = nc.dram_tensor("wqk", [D, 512], F32, kind="ExternalInput").ap()
    wv_d = nc.dram_tensor("wv", [D, 256], F32, kind="ExternalInput").ap()
    wo_d = nc.dram_tensor("wo", [256, D], F32, kind="ExternalInput").ap()
    patch_d = nc.dram_tensor("patch", [1, C, LH, 64], F16, kind="ExternalInput").ap()
    out_d = nc.dram_tensor("outT", [D, T2], F16, kind="ExternalOutput").ap()

    with (
        tc.tile_pool(name="const", bufs=1) as const,
        tc.tile_pool(name="p1", bufs=8) as p1pool,
        tc.tile_pool(name="rb", bufs=2) as rbpool,
        tc.tile_pool(name="psA", bufs=2, space="PSUM") as psA,
        tc.tile_pool(name="psB", bufs=2, space="PSUM") as psB,
    ):
        # ---- persistent SBUF tiles + input DMAs ----
        wqk = const.tile([128, D // 128, 512], MMDT)
        wqk_r = wqk_d.rearrange("(c p) n -> p c n", p=128).bitcast(MMDT)
        xt = const.tile([128, D // 128, T2], MMDT)
        xt_r = xt_d.rearrange("(c p) t -> p c t", p=128).bitcast(MMDT)
        c0, c1 = QCH[0]
        for dc in range(D // 128):
            nc.sync.dma_start(wqk[:, dc, :], wqk_r[:, dc, :])
            nc.scalar.dma_start(xt[:, dc, c0:c1], xt_r[:, dc, c0:c1])
        wv = const.tile([128, D // 128, 256], MMDT)
        nc.sync.dma_start(
            wv[:], wv_d.rearrange("(c p) n -> p c n", p=128).bitcast(MMDT))
        for c0, c1 in QCH[1:]:
            nc.sync.dma_start(xt[:, :, c0:c1], xt_r[:, :, c0:c1])
        wo = const.tile([128, 2, D], MMDT)
        nc.sync.dma_start(
            wo[:], wo_d.rearrange("(c p) n -> p c n", p=128).bitcast(MMDT))

        # V augmented: [128 keys, NVB, LH, 64 data + 64 ones]; denominator
        # rows (partition 0 of slot-leading blocks) carry (T - n_c - pads)
        v_sb = const.tile([128, NVB, LH, 128], F16)
        nc.vector.memset(v_sb[:, :, :, 64:128], 1.0)
        nc.sync.dma_start(v_sb[0:1, 0:C, :, 64:128], patch_d)

        # PE p-state warmup: zero matmuls bridge the initial DMA window so
        # real matmuls are priced at the ramped rate
        dummy = const.tile([128, 512], MMDT)
        nc.gpsimd.memset(dummy[:].bitcast(F32), 0.0)
        for wu in range(6):
            pswu = (psA, psB)[wu % 2].tile(
                [128, 512], F32, tag=("psA", "psB")[wu % 2], name=f"wu{wu}")
            nc.tensor.matmul(pswu[:, 0:512], lhsT=dummy[:, 0:128],
                             rhs=dummy[:, 0:512], start=True, stop=True)

        qtm = const.tile([128, 2, T2], F16)   # packed head pairs [h_even|h_odd]
        ktm = const.tile([128, 2, T2], F16)
        o_sc = const.tile([128, 2, T2], MMDT)
        used = As[-1] + Ws[-1]
        if used < T2:
            nc.vector.memset(o_sc[:, :, used:T2].bitcast(F32), 0.0)
        out_sb = const.tile([128, 4, T2], F16)

        def emit_qk(ci):
            c0, c1 = QCH[ci]
            W = c1 - c0
            for gi, (w_off, dst) in enumerate(((0, qtm), (256, ktm))):
                for hp in range(2):
                    pool_, tag = (psA, "psA") if (gi + hp) % 2 else (psB, "psB")
                    ps = pool_.tile([128, 512], F32, tag=tag, name=f"qk{ci}{gi}{hp}")
                    for dc in range(D // 128):
                        nc.tensor.matmul(
                            ps[:, :W],
                            lhsT=wqk[:, dc, w_off + hp * 128:w_off + (hp + 1) * 128],
                            rhs=xt[:, dc, c0:c1],
                            start=(dc == 0),
                            stop=(dc == D // 128 - 1),
                        )
                    cp = nc.scalar.copy if (gi + hp) % 2 else nc.vector.tensor_copy
                    cp(dst[:, hp, c0:c1], ps[:, :W])

        p1s = {}

        def emit_vs(s):
            """V blocks, score matmuls, and exp for slot s."""
            a, w = As[s], Ws[s]
            myk = [(klo, khi) for (ss, klo, khi) in kunits if ss == s]
            for vi, (klo, khi) in enumerate(myk):
                bw = khi - klo
                pool_, tag = (psA, "psA") if vi % 2 else (psB, "psB")
                psv = pool_.tile([128, 256], F32, tag=tag, name=f"v{s}_{klo}")
                for dc in range(D // 128):
                    nc.tensor.matmul(
                        psv[0:bw, 0:256],
                        lhsT=xt[:, dc, a + klo:a + khi],
                        rhs=wv[:, dc, 0:256],
                        start=(dc == 0),
                        stop=(dc == D // 128 - 1),
                    )
                nc.scalar.copy(
                    v_sb[0:bw, vblk[(s, klo)], :, 0:64],
                    psv[0:bw, 0:256].rearrange("p (h d) -> p h d", h=LH),
                )
            for ki, (klo, khi) in enumerate(myk):
                bw = khi - klo
                s_ps = psA.tile([128, LH, WPAD], F32, tag="psA", name=f"s{s}_{klo}")
                for j, h in enumerate(HO):
                    po = 64 * (h % 2)
                    nc.tensor.matmul(
                        s_ps[0:bw, j, 0:w],
                        lhsT=ktm[po:po + 64, h // 2, a + klo:a + khi],
                        rhs=qtm[po:po + 64, h // 2, a:a + w],
                        start=(j % 2 == 0),
                        stop=(j % 2 == 1),
                    )
                p1 = p1pool.tile([128, LH, WPAD], F16, tag="p1",
                                 name=f"p1_{s}_{klo}")
                nc.scalar.activation(
                    p1[0:bw, :, 0:w],
                    s_ps[0:bw, :, 0:w],
                    mybir.ActivationFunctionType.Exp,
                    scale=0.125,
                )
                p1s[(s, klo)] = p1

        def emit_pv(s, fast=False):
            a, w = As[s], Ws[s]
            osh = 0
            myk = [(klo, khi) for (ss, klo, khi) in kunits if ss == s]
            oaug = psB.tile([128, LH, WPAD], F32, tag="psB", name=f"oaug{s}")
            for ki, (klo, khi) in enumerate(myk):
                bw = khi - klo
                p1 = p1s.pop((s, klo))
                for j, h in enumerate(HO):
                    nc.tensor.matmul(
                        oaug[:, j, 0:w],
                        lhsT=v_sb[0:bw, vblk[(s, klo)], h, 0:128],
                        rhs=p1[0:bw, j, 0:w],
                        start=(j % 2 == 0 and ki == 0),
                        stop=(j % 2 == 1 and ki == len(myk) - 1),
                    )
            # normalize: o * (1/mass); slot order [0,2,1,3] makes each
            # partition-half one contiguous oaug slice
            recip = rbpool.tile([64, LH, WPAD], F32, tag="rc", name=f"rc{s}")
            if fast:
                # endgame: shortest latency to o_sc, PE has nothing to unblock
                nc.vector.reciprocal(
                    recip[:, :, 0:w], oaug[64:128, :, osh:osh + w])
                for pe in range(2):
                    nc.vector.tensor_mul(
                        o_sc[64 * pe:64 * pe + 64, :, a:a + w],
                        oaug[0:64, 2 * pe:2 * pe + 2, osh:osh + w],
                        recip[:, 2 * pe:2 * pe + 2, 0:w],
                    )
            else:
                # steady state: free the oaug psum early (one copy), then
                # normalize off-psum with muls on the idle Pool engine
                oc = rbpool.tile([128, LH, WPAD], F32, tag="oc", name=f"oc{s}")
                nc.vector.tensor_copy(oc[:, :, 0:w], oaug[:, :, osh:osh + w])
                nc.vector.reciprocal(recip[:, :, 0:w], oc[64:128, :, 0:w])
                for pe in range(2):
                    nc.gpsimd.tensor_mul(
                        o_sc[64 * pe:64 * pe + 64, :, a:a + w],
                        oc[0:64, 2 * pe:2 * pe + 2, 0:w],
                        recip[:, 2 * pe:2 * pe + 2, 0:w],
                    )

        def emit_outproj(ci):
            c0, c1 = QCH[ci]
            W = c1 - c0
            for doc in range(4):
                po = psA.tile([128, 512], F32, tag="psA", name=f"po{ci}_{doc}")
                for dhc in range(2):
                    nc.tensor.matmul(
                        po[:, :W],
                        lhsT=wo[:, dhc, doc * 128:(doc + 1) * 128],
                        rhs=o_sc[:, dhc, c0:c1],
                        start=(dhc == 0),
                        stop=(dhc == 1),
                    )
                cp = nc.scalar.copy if doc % 2 else nc.vector.tensor_copy
                cp(out_sb[:, doc, c0:c1], po[:, :W])
            nc.sync.dma_start(
                out_d.rearrange("(c p) t -> p c t", p=128)[:, :, c0:c1],
                out_sb[:, :, c0:c1],
            )

        # out-proj chunk ci is ready after the last slot whose [a, a+w)
        # intersects its columns has been normalized
        oready = []
        for c0, c1 in QCH:
            oready.append(max(s for s in range(C) if As[s] < c1))

        # ---- software-pipelined schedule ----
        ready = []            # slots with v+score+exp emitted, pv pending
        normed = -1

        opending = list(range(len(QCH)))

        def pv_front():
            nonlocal normed
            s = ready.pop(0)
            emit_pv(s, fast=(s >= C - 2))
            normed = s
            while opending and oready[opending[0]] < s:
                emit_outproj(opending.pop(0))

        emit_qk(0)
        for ci in range(len(QCH)):
            if ci > 0:
                emit_qk(ci)
            for s in sgrp[ci]:
                emit_vs(s)
                ready.append(s)
                if len(ready) >= 4:
                    pv_front()
        while ready:
            pv_front()
        while opending:
            emit_outproj(opending.pop(0))


def build_nc(T2, Ws, As):
    nc = bacc.Bacc("TRN2", target_bir_lowering=False, debug=False, num_devices=8)
    with tile.TileContext(nc) as tc:
        _kernel_body(tc, T2, Ws, As)
    nc.compile()
    return nc


def prepare(X, Wc, bc, Win, Wout):
    """Host-side clustering, canonical layout, and per-core input maps."""
    X = np.asarray(X, np.float32)
    Wc = np.asarray(Wc, np.float32)
    bc = np.asarray(bc, np.float32)
    Win = np.asarray(Win, np.float32)
    Wout = np.asarray(Wout, np.float32)

    assign_all = np.stack(
        [(X[b] @ Wc.T + bc).argmax(-1) for b in range(B)]
    )
    T2, Ws, As = make_schedule(assign_all)
    order = np.argsort(
        -np.stack([np.bincount(a, minlength=C) for a in assign_all]),
        axis=1, kind="stable")

    per_batch = []
    poss = []
    for b in range(B):
        a = assign_all[b]
        X2 = np.zeros((T2, D), np.float32)
        pos = np.empty(T, np.int64)
        patch = np.empty((C, LH, 64), np.float32)
        for s in range(C):
            c = order[b, s]
            toks = np.nonzero(a == c)[0]
            n = len(toks)
            A = As[s]
            pad_cnt = Ws[s] - 1 - n
            patch[s] = float(T - n - pad_cnt)
            X2[A + 1:A + 1 + n] = X[b, toks]
            pos[toks] = np.arange(A + 1, A + 1 + n)
        per_batch.append(
            {
                "xt": np.ascontiguousarray(X2.T),
                "patch": patch[None].astype(np.float16),
            }
        )
        poss.append(pos)

    per_half = []
    for hh in range(2):
        r = slice(hh * 256, (hh + 1) * 256)
        wqk = np.concatenate([Win[0:D][r].T, Win[D:2 * D][r].T], axis=1)
        per_half.append(
            {
                "wqk": np.ascontiguousarray(wqk),
                "wv": np.ascontiguousarray(Win[2 * D:][r].T),
                "wo": np.ascontiguousarray(Wout[:, r].T),
            }
        )

    in_maps = [dict(per_batch[g // 2], **per_half[g % 2]) for g in range(8)]
    return (T2, tuple(Ws), tuple(As)), in_maps, poss


_NC_CACHE = {}


def kernel(X, Wc, bc, Win, bin_, Wout, bout):
    assert not np.any(np.asarray(bin_)), "kernel assumes zero in_proj bias"
    sched, in_maps, poss = prepare(X, Wc, bc, Win, Wout)
    if sched not in _NC_CACHE:
        _NC_CACHE[sched] = build_nc(sched[0], list(sched[1]), list(sched[2]))
    nc = _NC_CACHE[sched]
    res = run_bass_kernel_spmd(nc, in_maps, core_ids=list(range(8)))
    outs = res.results
    bout = np.asarray(bout, np.float32)
    out = np.empty((B, T, D), np.float32)
    for b in range(B):
        full = outs[2 * b]["outT"].astype(np.float32) + \
            outs[2 * b + 1]["outT"].astype(np.float32)
        out[b] = full.T[poss[b]] + bout
    return out


# revision 3
# speedup vs baseline: 1.0061x; 1.0061x over previous
"""Trainium2 Bass kernel v2.2 for NeuralClusteringAttention.

Tokens are sorted by cluster on host -> attention is exactly block-diagonal
per cluster slot: keys for a slot's queries are a column window of K and a
partition block of V, so no mask rows are needed.

Reference softmax semantics (masked scores are 0, not -inf) are reproduced
with a zero-X "denominator token" per slot whose V ones-columns carry weight
(T - n_c - pad_cnt): pad keys contribute exp(0)*1 to the mass and 0 to
values; the reduced denominator weight cancels them exactly.

Everything runs in float32r: self-loading weights (no per-matmul Ldweights
on PE.SEQ) at 1 cycle/col for >=256-wide moving dims. Attention matmuls pad
their moving dim to 256 (T2 is extended so reads stay in bounds); the exp
only covers the real slot width, and each p1 tile's pad columns are
zero-memset on the idle Pool engine so the padded PV reads stay finite.
Heads are ordered [0,2,1,3] in the score/PV psum so each psum bank's
accumulation group keeps a uniform PE tile_position.

Emission is software-pipelined: score(s+1) issues on PE before PV(s), so
exp(s) latency on ACT hides behind real PE work, and each engine queue's
in-order stream matches data-readiness order.

Sharding: 8 cores = (4 batches) x (2 head-halves of 4 heads each). Host
sums the two fp16 partials per batch and adds bout.
"""

import numpy as np

import concourse.bacc as bacc
import concourse.bass as bass
import concourse.mybir as mybir
import concourse.tile as tile
from concourse.bass_utils import run_bass_kernel_spmd

B, T, D, C, H = 4, 1024, 512, 8, 8
HD = D // H          # 64
LH = 4               # local heads per core
F32 = mybir.dt.float32
F16 = mybir.dt.float16
MMDT = mybir.dt.float32r
HO = (0, 2, 1, 3)    # head slot order: uniform tile_position per psum bank
WPAD = 256           # attention moving-dim pad (fp32r full rate at >=256)


def make_schedule(assign_all):
    """Canonical per-slot layout shared by all batches (single SPMD program)."""
    counts = np.stack([np.bincount(a, minlength=C) for a in assign_all])
    sizes = -np.sort(-counts, axis=1)                      # [B, C] desc
    Ws = ((sizes.max(axis=0) + 1 + 3) // 4) * 4            # canonical widths
    As = np.zeros(C, np.int64)
    As[1:] = np.cumsum(Ws)[:-1]
    used = int(As[-1] + Ws[-1])
    T2 = ((used + 15) // 16) * 16
    return T2, [int(w) for w in Ws], [int(a) for a in As]


def _chunks(T2):
    """Column chunks for qk/out/xt: <=512 (one psum bank) and >=256 wide
    (fp32r full rate), last two balanced."""
    ch = []
    o = 0
    while o < T2:
        rem = T2 - o
        w = rem if rem <= 512 else (512 if rem >= 768 else (rem + 1) // 2 // 8 * 8)
        ch.append((o, o + w))
        o += w
    return ch


def _kernel_body(tc, T2, Ws, As):
    nc = tc.nc
    QCH = _chunks(T2)
    assert max(Ws) <= WPAD
    # key blocks per slot
    kunits = []
    for s, (a, w) in enumerate(zip(As, Ws)):
        kunits.append((s, 0, min(w, 128)))
        if w > 128:
            kunits.append((s, 128, w))
    # v block index: slot-leading blocks 0..7 so the denominator patch is one
    # regular-stride DMA; overflow blocks follow
    vblk = {}
    novf = 0
    for s, klo, khi in kunits:
        if klo == 0:
            vblk[(s, 0)] = s
        else:
            vblk[(s, klo)] = C + novf
            novf += 1
    NVB = C + novf

    # slot groups by the qk chunk their padded score window needs
    ends = [min(a + WPAD, T2) for a in As]  # shifted windows end here
    sgrp = [[] for _ in QCH]
    for s in range(C):
        ci = next(i for i, (c0, c1) in enumerate(QCH) if ends[s] <= c1)
        sgrp[ci].append(s)

    xt_d = nc.dram_tensor("xt", [D, T2], F32, kind="ExternalInput").ap()
    wqk_d = nc.dram_tensor("wqk", [D, 512], F32, kind="ExternalInput").ap()
    wv_d = nc.dram_tensor("wv", [D, 256], F32, kind="ExternalInput").ap()
    wo_d = nc.dram_tensor("wo", [256, D], F32, kind="ExternalInput").ap()
    patch_d = nc.dram_tensor("patch", [1, C, LH, 64], F16, kind="ExternalInput").ap()
    out_d = nc.dram_tensor("outT", [D, T2], F16, kind="ExternalOutput").ap()

    with (
        tc.tile_pool(name="const", bufs=1) as const,
        tc.tile_pool(name="p1", bufs=10) as p1pool,
        tc.tile_pool(name="rb", bufs=2) as rbpool,
        tc.tile_pool(name="psA", bufs=2, space="PSUM") as psA,
        tc.tile_pool(name="psB", bufs=2, space="PSUM") as psB,
    ):
        # ---- persistent SBUF tiles + input DMAs ----
        wqk = const.tile([128, D // 128, 512], MMDT)
        wqk_r = wqk_d.rearrange("(c p) n -> p c n", p=128).bitcast(MMDT)
        xt = const.tile([128, D // 128, T2], MMDT)
        xt_r = xt_d.rearrange("(c p) t -> p c t", p=128).bitcast(MMDT)
        c0, c1 = QCH[0]
        for dc in range(D // 128):
            nc.sync.dma_start(wqk[:, dc, :], wqk_r[:, dc, :])
            nc.scalar.dma_start(xt[:, dc, c0:c1], xt_r[:, dc, c0:c1])
        wv = const.tile([128, D // 128, 256], MMDT)
        nc.sync.dma_start(
            wv[:], wv_d.rearrange("(c p) n -> p c n", p=128).bitcast(MMDT))
        for c0, c1 in QCH[1:]:
            nc.sync.dma_start(xt[:, :, c0:c1], xt_r[:, :, c0:c1])
        wo = const.tile([128, 2, D], MMDT)
        nc.sync.dma_start(
            wo[:], wo_d.rearrange("(c p) n -> p c n", p=128).bitcast(MMDT))

        # V augmented: [128 keys, NVB, LH, 64 data + 64 ones]; denominator
        # rows (partition 0 of slot-leading blocks) carry (T - n_c - pads)
        v_sb = const.tile([128, NVB, LH, 128], F16)
        nc.vector.memset(v_sb[:, :, :, 64:128], 1.0)
        nc.sync.dma_start(v_sb[0:1, 0:C, :, 64:128], patch_d)

        # PE p-state warmup: zero matmuls bridge the initial DMA window so
        # real matmuls are priced at the ramped rate
        dummy = const.tile([128, 512], MMDT)
        nc.gpsimd.memset(dummy[:].bitcast(F32), 0.0)
        for wu in range(4):
            pswu = (psA, psB)[wu % 2].tile(
                [128, 512], F32, tag=("psA", "psB")[wu % 2], name=f"wu{wu}")
            nc.tensor.matmul(pswu[:, 0:512], lhsT=dummy[:, 0:128],
                             rhs=dummy[:, 0:512], start=True, stop=True)

        qtm = const.tile([128, 2, T2], F16)   # packed head pairs [h_even|h_odd]
        ktm = const.tile([128, 2, T2], F16)
        o_sc = const.tile([128, 2, T2], MMDT)
        used = As[-1] + Ws[-1]
        if used < T2:
            nc.vector.memset(o_sc[:, :, used:T2].bitcast(F32), 0.0)
        out_sb = const.tile([128, 4, T2], F16)

        def emit_qk(ci):
            c0, c1 = QCH[ci]
            W = c1 - c0
            for gi, (w_off, dst) in enumerate(((0, qtm), (256, ktm))):
                for hp in range(2):
                    pool_, tag = (psA, "psA") if (gi + hp) % 2 else (psB, "psB")
                    ps = pool_.tile([128, 512], F32, tag=tag, name=f"qk{ci}{gi}{hp}")
                    for dc in range(D // 128):
                        nc.tensor.matmul(
                            ps[:, :W],
                            lhsT=wqk[:, dc, w_off + hp * 128:w_off + (hp + 1) * 128],
                            rhs=xt[:, dc, c0:c1],
                            start=(dc == 0),
                            stop=(dc == D // 128 - 1),
                        )
                    cp = nc.scalar.copy if (gi + hp) % 2 else nc.vector.tensor_copy
                    cp(dst[:, hp, c0:c1], ps[:, :W])

        p1s = {}

        def emit_vs(s):
            """V blocks, score matmuls, and exp for slot s."""
            a, w = As[s], Ws[s]
            myk = [(klo, khi) for (ss, klo, khi) in kunits if ss == s]
            for vi, (klo, khi) in enumerate(myk):
                bw = khi - klo
                pool_, tag = (psA, "psA") if vi % 2 else (psB, "psB")
                psv = pool_.tile([128, 256], F32, tag=tag, name=f"v{s}_{klo}")
                for dc in range(D // 128):
                    nc.tensor.matmul(
                        psv[0:bw, 0:256],
                        lhsT=xt[:, dc, a + klo:a + khi],
                        rhs=wv[:, dc, 0:256],
                        start=(dc == 0),
                        stop=(dc == D // 128 - 1),
                    )
                nc.scalar.copy(
                    v_sb[0:bw, vblk[(s, klo)], :, 0:64],
                    psv[0:bw, 0:256].rearrange("p (h d) -> p h d", h=LH),
                )
            for ki, (klo, khi) in enumerate(myk):
                bw = khi - klo
                s_ps = psA.tile([128, LH, WPAD], F32, tag="psA", name=f"s{s}_{klo}")
                for j, h in enumerate(HO):
                    po = 64 * (h % 2)
                    nc.tensor.matmul(
                        s_ps[0:bw, j, 0:w],
                        lhsT=ktm[po:po + 64, h // 2, a + klo:a + khi],
                        rhs=qtm[po:po + 64, h // 2, a:a + w],
                        start=(j % 2 == 0),
                        stop=(j % 2 == 1),
                    )
                p1 = p1pool.tile([128, LH, WPAD], F16, tag="p1",
                                 name=f"p1_{s}_{klo}")
                nc.scalar.activation(
                    p1[0:bw, :, 0:w],
                    s_ps[0:bw, :, 0:w],
                    mybir.ActivationFunctionType.Exp,
                    scale=0.125,
                )
                p1s[(s, klo)] = p1

        def emit_pv(s, fast=False):
            a, w = As[s], Ws[s]
            osh = 0
            myk = [(klo, khi) for (ss, klo, khi) in kunits if ss == s]
            oaug = psB.tile([128, LH, WPAD], F32, tag="psB", name=f"oaug{s}")
            for ki, (klo, khi) in enumerate(myk):
                bw = khi - klo
                p1 = p1s.pop((s, klo))
                for j, h in enumerate(HO):
                    nc.tensor.matmul(
                        oaug[:, j, 0:w],
                        lhsT=v_sb[0:bw, vblk[(s, klo)], h, 0:128],
                        rhs=p1[0:bw, j, 0:w],
                        start=(j % 2 == 0 and ki == 0),
                        stop=(j % 2 == 1 and ki == len(myk) - 1),
                    )
            # normalize: o * (1/mass); slot order [0,2,1,3] makes each
            # partition-half one contiguous oaug slice
            recip = rbpool.tile([64, LH, WPAD], F32, tag="rc", name=f"rc{s}")
            if fast:
                # endgame: shortest latency to o_sc, PE has nothing to unblock
                nc.vector.reciprocal(
                    recip[:, :, 0:w], oaug[64:128, :, osh:osh + w])
                for pe in range(2):
                    nc.vector.tensor_mul(
                        o_sc[64 * pe:64 * pe + 64, :, a:a + w],
                        oaug[0:64, 2 * pe:2 * pe + 2, osh:osh + w],
                        recip[:, 2 * pe:2 * pe + 2, 0:w],
                    )
            else:
                # steady state: free the oaug psum early (one copy), then
                # normalize off-psum with muls on the idle Pool engine
                oc = rbpool.tile([128, LH, WPAD], F32, tag="oc", name=f"oc{s}")
                nc.vector.tensor_copy(oc[:, :, 0:w], oaug[:, :, osh:osh + w])
                nc.vector.reciprocal(recip[:, :, 0:w], oc[64:128, :, 0:w])
                for pe in range(2):
                    nc.gpsimd.tensor_mul(
                        o_sc[64 * pe:64 * pe + 64, :, a:a + w],
                        oc[0:64, 2 * pe:2 * pe + 2, 0:w],
                        recip[:, 2 * pe:2 * pe + 2, 0:w],
                    )

        def emit_outproj(ci):
            c0, c1 = QCH[ci]
            W = c1 - c0
            for doc in range(4):
                po = psA.tile([128, 512], F32, tag="psA", name=f"po{ci}_{doc}")
                for dhc in range(2):
                    nc.tensor.matmul(
                        po[:, :W],
                        lhsT=wo[:, dhc, doc * 128:(doc + 1) * 128],
                        rhs=o_sc[:, dhc, c0:c1],
                        start=(dhc == 0),
                        stop=(dhc == 1),
                    )
                cp = nc.scalar.copy if doc % 2 else nc.vector.tensor_copy
                cp(out_sb[:, doc, c0:c1], po[:, :W])
            nc.sync.dma_start(
                out_d.rearrange("(c p) t -> p c t", p=128)[:, :, c0:c1],
                out_sb[:, :, c0:c1],
            )

        # out-proj chunk ci is ready after the last slot whose [a, a+w)
        # intersects its columns has been normalized
        oready = []
        for c0, c1 in QCH:
            oready.append(max(s for s in range(C) if As[s] < c1))

        # ---- software-pipelined schedule ----
        ready = []            # slots with v+score+exp emitted, pv pending
        normed = -1

        opending = list(range(len(QCH)))

        def pv_front():
            nonlocal normed
            s = ready.pop(0)
            emit_pv(s, fast=(s >= C - 2))
            normed = s
            while opending and oready[opending[0]] < s:
                emit_outproj(opending.pop(0))

        emit_qk(0)
        for ci in range(len(QCH)):
            if ci > 0:
                emit_qk(ci)
            for s in sgrp[ci]:
                emit_vs(s)
                ready.append(s)
                if len(ready) >= 5:
                    pv_front()
        while ready:
            pv_front()
        while opending:
            emit_outproj(opending.pop(0))


def build_nc(T2, Ws, As):
    nc = bacc.Bacc("TRN2", target_bir_lowering=False, debug=False, num_devices=8)
    with tile.TileContext(nc) as tc:
        _kernel_body(tc, T2, Ws, As)
    nc.compile()
    return nc


def prepare(X, Wc, bc, Win, Wout):
    """Host-side clustering, canonical layout, and per-core input maps."""
    X = np.asarray(X, np.float32)
    Wc = np.asarray(Wc, np.float32)
    bc = np.asarray(bc, np.float32)
    Win = np.asarray(Win, np.float32)
    Wout = np.asarray(Wout, np.float32)

    assign_all = np.stack(
        [(X[b] @ Wc.T + bc).argmax(-1) for b in range(B)]
    )
    T2, Ws, As = make_schedule(assign_all)
    order = np.argsort(
        -np.stack([np.bincount(a, minlength=C) for a in assign_all]),
        axis=1, kind="stable")

    per_batch = []
    poss = []
    for b in range(B):
        a = assign_all[b]
        X2 = np.zeros((T2, D), np.float32)
        pos = np.empty(T, np.int64)
        patch = np.empty((C, LH, 64), np.float32)
        for s in range(C):
            c = order[b, s]
            toks = np.nonzero(a == c)[0]
            n = len(toks)
            A = As[s]
            pad_cnt = Ws[s] - 1 - n
            patch[s] = float(T - n - pad_cnt)
            X2[A + 1:A + 1 + n] = X[b, toks]
            pos[toks] = np.arange(A + 1, A + 1 + n)
        per_batch.append(
            {
                "xt": np.ascontiguousarray(X2.T),
                "patch": patch[None].astype(np.float16),
            }
        )
        poss.append(pos)

    per_half = []
    for hh in range(2):
        r = slice(hh * 256, (hh + 1) * 256)
        wqk = np.concatenate([Win[0:D][r].T, Win[D:2 * D][r].T], axis=1)
        per_half.append(
            {
                "wqk": np.ascontiguousarray(wqk),
                "wv": np.ascontiguousarray(Win[2 * D:][r].T),
                "wo": np.ascontiguousarray(Wout[:, r].T),
            }
        )

    in_maps = [dict(per_batch[g // 2], **per_half[g % 2]) for g in range(8)]
    return (T2, tuple(Ws), tuple(As)), in_maps, poss


_NC_CACHE = {}


def kernel(X, Wc, bc, Win, bin_, Wout, bout):
    assert not np.any(np.asarray(bin_)), "kernel assumes zero in_proj bias"
    sched, in_maps, poss = prepare(X, Wc, bc, Win, Wout)
    if sched not in _NC_CACHE:
        _NC_CACHE[sched] = build_nc(sched[0], list(sched[1]), list(sched[2]))
    nc = _NC_CACHE[sched]
    res = run_bass_kernel_spmd(nc, in_maps, core_ids=list(range(8)))
    outs = res.results
    bout = np.asarray(bout, np.float32)
    out = np.empty((B, T, D), np.float32)
    for b in range(B):
        full = outs[2 * b]["outT"].astype(np.float32) + \
            outs[2 * b + 1]["outT"].astype(np.float32)
        out[b] = full.T[poss[b]] + bout
    return out


# revision 4
# speedup vs baseline: 1.0111x; 1.0051x over previous
"""Trainium2 Bass kernel v2.2 for NeuralClusteringAttention.

Tokens are sorted by cluster on host -> attention is exactly block-diagonal
per cluster slot: keys for a slot's queries are a column window of K and a
partition block of V, so no mask rows are needed.

Reference softmax semantics (masked scores are 0, not -inf) are reproduced
with a zero-X "denominator token" per slot whose V ones-columns carry weight
(T - n_c - pad_cnt): pad keys contribute exp(0)*1 to the mass and 0 to
values; the reduced denominator weight cancels them exactly.

Everything runs in float32r: self-loading weights (no per-matmul Ldweights
on PE.SEQ) at 1 cycle/col for >=256-wide moving dims. Attention matmuls pad
their moving dim to 256 (T2 is extended so reads stay in bounds); the exp
only covers the real slot width, and each p1 tile's pad columns are
zero-memset on the idle Pool engine so the padded PV reads stay finite.
Heads are ordered [0,2,1,3] in the score/PV psum so each psum bank's
accumulation group keeps a uniform PE tile_position.

Emission is software-pipelined: score(s+1) issues on PE before PV(s), so
exp(s) latency on ACT hides behind real PE work, and each engine queue's
in-order stream matches data-readiness order.

Sharding: 8 cores = (4 batches) x (2 head-halves of 4 heads each). Host
sums the two fp16 partials per batch and adds bout.
"""

import numpy as np

import concourse.bacc as bacc
import concourse.bass as bass
import concourse.mybir as mybir
import concourse.tile as tile
from concourse.bass_utils import run_bass_kernel_spmd

B, T, D, C, H = 4, 1024, 512, 8, 8
HD = D // H          # 64
LH = 4               # local heads per core
F32 = mybir.dt.float32
F16 = mybir.dt.float16
MMDT = mybir.dt.float32r
HO = (0, 2, 1, 3)    # head slot order: uniform tile_position per psum bank
WPAD = 256           # attention moving-dim pad (fp32r full rate at >=256)


def make_schedule(assign_all):
    """Canonical per-slot layout shared by all batches (single SPMD program)."""
    counts = np.stack([np.bincount(a, minlength=C) for a in assign_all])
    sizes = -np.sort(-counts, axis=1)                      # [B, C] desc
    Ws = ((sizes.max(axis=0) + 1 + 3) // 4) * 4            # canonical widths
    As = np.zeros(C, np.int64)
    As[1:] = np.cumsum(Ws)[:-1]
    used = int(As[-1] + Ws[-1])
    T2 = ((used + 15) // 16) * 16
    return T2, [int(w) for w in Ws], [int(a) for a in As]


def _chunks(T2):
    """Column chunks for qk/out/xt: <=512 (one psum bank) and >=256 wide
    (fp32r full rate), last two balanced."""
    ch = []
    o = 0
    while o < T2:
        rem = T2 - o
        w = rem if rem <= 512 else (512 if rem >= 768 else (rem + 1) // 2 // 8 * 8)
        ch.append((o, o + w))
        o += w
    return ch


def _kernel_body(tc, T2, Ws, As):
    nc = tc.nc
    QCH = _chunks(T2)
    assert max(Ws) <= WPAD
    # key blocks per slot
    kunits = []
    for s, (a, w) in enumerate(zip(As, Ws)):
        kunits.append((s, 0, min(w, 128)))
        if w > 128:
            kunits.append((s, 128, w))
    # v block index: slot-leading blocks 0..7 so the denominator patch is one
    # regular-stride DMA; overflow blocks follow
    vblk = {}
    novf = 0
    for s, klo, khi in kunits:
        if klo == 0:
            vblk[(s, 0)] = s
        else:
            vblk[(s, klo)] = C + novf
            novf += 1
    NVB = C + novf

    # slot groups by the qk chunk their padded score window needs
    ends = [min(a + WPAD, T2) for a in As]  # shifted windows end here
    sgrp = [[] for _ in QCH]
    for s in range(C):
        ci = next(i for i, (c0, c1) in enumerate(QCH) if ends[s] <= c1)
        sgrp[ci].append(s)

    xt_d = nc.dram_tensor("xt", [D, T2], F32, kind="ExternalInput").ap()
    wqk_d = nc.dram_tensor("wqk", [D, 512], F32, kind="ExternalInput").ap()
    wv_d = nc.dram_tensor("wv", [D, 256], F32, kind="ExternalInput").ap()
    wo_d = nc.dram_tensor("wo", [256, D], F32, kind="ExternalInput").ap()
    patch_d = nc.dram_tensor("patch", [1, C, LH, 64], F16, kind="ExternalInput").ap()
    out_d = nc.dram_tensor("outT", [D, T2], F16, kind="ExternalOutput").ap()

    with (
        tc.tile_pool(name="const", bufs=1) as const,
        tc.tile_pool(name="p1", bufs=10) as p1pool,
        tc.tile_pool(name="rb", bufs=2) as rbpool,
        tc.tile_pool(name="psA", bufs=2, space="PSUM") as psA,
        tc.tile_pool(name="psB", bufs=2, space="PSUM") as psB,
    ):
        # ---- persistent SBUF tiles + input DMAs ----
        wqk = const.tile([128, D // 128, 512], MMDT)
        wqk_r = wqk_d.rearrange("(c p) n -> p c n", p=128).bitcast(MMDT)
        xt = const.tile([128, D // 128, T2], MMDT)
        xt_r = xt_d.rearrange("(c p) t -> p c t", p=128).bitcast(MMDT)
        c0, c1 = QCH[0]
        for dc in range(D // 128):
            nc.sync.dma_start(wqk[:, dc, :], wqk_r[:, dc, :])
            nc.scalar.dma_start(xt[:, dc, c0:c1], xt_r[:, dc, c0:c1])
        wv = const.tile([128, D // 128, 256], MMDT)
        nc.sync.dma_start(
            wv[:], wv_d.rearrange("(c p) n -> p c n", p=128).bitcast(MMDT))
        for c0, c1 in QCH[1:]:
            nc.sync.dma_start(xt[:, :, c0:c1], xt_r[:, :, c0:c1])
        wo = const.tile([128, 2, D], MMDT)
        nc.sync.dma_start(
            wo[:], wo_d.rearrange("(c p) n -> p c n", p=128).bitcast(MMDT))

        # V augmented: [128 keys, NVB, LH, 64 data + 64 ones]; denominator
        # rows (partition 0 of slot-leading blocks) carry (T - n_c - pads)
        v_sb = const.tile([128, NVB, LH, 128], F16)
        nc.vector.memset(v_sb[:, :, :, 64:128], 1.0)
        nc.sync.dma_start(v_sb[0:1, 0:C, :, 64:128], patch_d)

        # PE p-state warmup: zero matmuls bridge the initial DMA window so
        # real matmuls are priced at the ramped rate
        dummy = const.tile([128, 512], MMDT)
        nc.gpsimd.memset(dummy[:].bitcast(F32), 0.0)
        for wu in range(4):
            pswu = (psA, psB)[wu % 2].tile(
                [128, 512], F32, tag=("psA", "psB")[wu % 2], name=f"wu{wu}")
            nc.tensor.matmul(pswu[:, 0:512], lhsT=dummy[:, 0:128],
                             rhs=dummy[:, 0:512], start=True, stop=True)

        qtm = const.tile([128, 2, T2], F16)   # packed head pairs [h_even|h_odd]
        ktm = const.tile([128, 2, T2], F16)
        o_sc = const.tile([128, 2, T2], MMDT)
        used = As[-1] + Ws[-1]
        if used < T2:
            nc.vector.memset(o_sc[:, :, used:T2].bitcast(F32), 0.0)
        out_sb = const.tile([128, 4, T2], F16)

        def emit_qk(ci):
            c0, c1 = QCH[ci]
            W = c1 - c0
            for gi, (w_off, dst) in enumerate(((0, qtm), (256, ktm))):
                for hp in range(2):
                    pool_, tag = (psA, "psA") if (gi + hp) % 2 else (psB, "psB")
                    ps = pool_.tile([128, 512], F32, tag=tag, name=f"qk{ci}{gi}{hp}")
                    for dc in range(D // 128):
                        nc.tensor.matmul(
                            ps[:, :W],
                            lhsT=wqk[:, dc, w_off + hp * 128:w_off + (hp + 1) * 128],
                            rhs=xt[:, dc, c0:c1],
                            start=(dc == 0),
                            stop=(dc == D // 128 - 1),
                        )
                    cp = nc.scalar.copy if (gi + hp) % 2 else nc.vector.tensor_copy
                    cp(dst[:, hp, c0:c1], ps[:, :W])

        p1s = {}

        def emit_vs(s):
            """V blocks, score matmuls, and exp for slot s."""
            a, w = As[s], Ws[s]
            myk = [(klo, khi) for (ss, klo, khi) in kunits if ss == s]
            for vi, (klo, khi) in enumerate(myk):
                bw = khi - klo
                pool_, tag = (psA, "psA") if vi % 2 else (psB, "psB")
                psv = pool_.tile([128, 256], F32, tag=tag, name=f"v{s}_{klo}")
                for dc in range(D // 128):
                    nc.tensor.matmul(
                        psv[0:bw, 0:256],
                        lhsT=xt[:, dc, a + klo:a + khi],
                        rhs=wv[:, dc, 0:256],
                        start=(dc == 0),
                        stop=(dc == D // 128 - 1),
                    )
                nc.scalar.copy(
                    v_sb[0:bw, vblk[(s, klo)], :, 0:64],
                    psv[0:bw, 0:256].rearrange("p (h d) -> p h d", h=LH),
                )
            for ki, (klo, khi) in enumerate(myk):
                bw = khi - klo
                s_ps = psA.tile([128, LH, WPAD], F32, tag="psA", name=f"s{s}_{klo}")
                for j, h in enumerate(HO):
                    po = 64 * (h % 2)
                    nc.tensor.matmul(
                        s_ps[0:bw, j, 0:w],
                        lhsT=ktm[po:po + 64, h // 2, a + klo:a + khi],
                        rhs=qtm[po:po + 64, h // 2, a:a + w],
                        start=(j % 2 == 0),
                        stop=(j % 2 == 1),
                    )
                p1 = p1pool.tile([128, LH, WPAD], F16, tag="p1",
                                 name=f"p1_{s}_{klo}")
                nc.scalar.activation(
                    p1[0:bw, :, 0:w],
                    s_ps[0:bw, :, 0:w],
                    mybir.ActivationFunctionType.Exp,
                    scale=0.125,
                )
                p1s[(s, klo)] = p1

        def emit_pv(s, fast=False):
            a, w = As[s], Ws[s]
            osh = 0
            myk = [(klo, khi) for (ss, klo, khi) in kunits if ss == s]
            oaug = psB.tile([128, LH, WPAD], F32, tag="psB", name=f"oaug{s}")
            for ki, (klo, khi) in enumerate(myk):
                bw = khi - klo
                p1 = p1s.pop((s, klo))
                for j, h in enumerate(HO):
                    nc.tensor.matmul(
                        oaug[:, j, 0:w],
                        lhsT=v_sb[0:bw, vblk[(s, klo)], h, 0:128],
                        rhs=p1[0:bw, j, 0:w],
                        start=(j % 2 == 0 and ki == 0),
                        stop=(j % 2 == 1 and ki == len(myk) - 1),
                    )
            # normalize: o * (1/mass); slot order [0,2,1,3] makes each
            # partition-half one contiguous oaug slice
            recip = rbpool.tile([64, LH, WPAD], F32, tag="rc", name=f"rc{s}")
            if fast:
                # endgame: shortest latency to o_sc, PE has nothing to unblock
                nc.vector.reciprocal(
                    recip[:, :, 0:w], oaug[64:128, :, osh:osh + w])
                for pe in range(2):
                    nc.vector.tensor_mul(
                        o_sc[64 * pe:64 * pe + 64, :, a:a + w],
                        oaug[0:64, 2 * pe:2 * pe + 2, osh:osh + w],
                        recip[:, 2 * pe:2 * pe + 2, 0:w],
                    )
            else:
                # steady state: free the oaug psum early (one copy), then
                # normalize off-psum with muls on the idle Pool engine
                oc = rbpool.tile([128, LH, WPAD], F32, tag="oc", name=f"oc{s}")
                nc.vector.tensor_copy(oc[:, :, 0:w], oaug[:, :, osh:osh + w])
                nc.vector.reciprocal(recip[:, :, 0:w], oc[64:128, :, 0:w])
                for pe in range(2):
                    nc.gpsimd.tensor_mul(
                        o_sc[64 * pe:64 * pe + 64, :, a:a + w],
                        oc[0:64, 2 * pe:2 * pe + 2, 0:w],
                        recip[:, 2 * pe:2 * pe + 2, 0:w],
                    )

        def emit_outproj(ci):
            c0, c1 = QCH[ci]
            W = c1 - c0
            for doc in range(4):
                po = psA.tile([128, 512], F32, tag="psA", name=f"po{ci}_{doc}")
                for dhc in range(2):
                    nc.tensor.matmul(
                        po[:, :W],
                        lhsT=wo[:, dhc, doc * 128:(doc + 1) * 128],
                        rhs=o_sc[:, dhc, c0:c1],
                        start=(dhc == 0),
                        stop=(dhc == 1),
                    )
                cp = nc.scalar.copy if doc % 2 else nc.vector.tensor_copy
                cp(out_sb[:, doc, c0:c1], po[:, :W])
                if ci == len(QCH) - 1 and doc % 2:
                    # tail: ship doc pairs as soon as both copies land
                    nc.sync.dma_start(
                        out_d.rearrange("(c p) t -> p c t", p=128)[
                            :, doc - 1:doc + 1, c0:c1],
                        out_sb[:, doc - 1:doc + 1, c0:c1],
                    )
            if ci != len(QCH) - 1:
                nc.sync.dma_start(
                    out_d.rearrange("(c p) t -> p c t", p=128)[:, :, c0:c1],
                    out_sb[:, :, c0:c1],
                )

        # out-proj chunk ci is ready after the last slot whose [a, a+w)
        # intersects its columns has been normalized
        oready = []
        for c0, c1 in QCH:
            oready.append(max(s for s in range(C) if As[s] < c1))

        # ---- software-pipelined schedule ----
        ready = []            # slots with v+score+exp emitted, pv pending
        normed = -1

        opending = list(range(len(QCH)))

        def pv_front():
            nonlocal normed
            s = ready.pop(0)
            emit_pv(s, fast=(s >= C - 2))
            normed = s
            while opending and oready[opending[0]] < s:
                emit_outproj(opending.pop(0))

        emit_qk(0)
        for ci in range(len(QCH)):
            if ci > 0:
                emit_qk(ci)
            for s in sgrp[ci]:
                emit_vs(s)
                ready.append(s)
                if len(ready) >= 5:
                    pv_front()
        while ready:
            pv_front()
        while opending:
            emit_outproj(opending.pop(0))


def build_nc(T2, Ws, As):
    nc = bacc.Bacc("TRN2", target_bir_lowering=False, debug=False, num_devices=8)
    with tile.TileContext(nc) as tc:
        _kernel_body(tc, T2, Ws, As)
    nc.compile()
    return nc


def prepare(X, Wc, bc, Win, Wout):
    """Host-side clustering, canonical layout, and per-core input maps."""
    X = np.asarray(X, np.float32)
    Wc = np.asarray(Wc, np.float32)
    bc = np.asarray(bc, np.float32)
    Win = np.asarray(Win, np.float32)
    Wout = np.asarray(Wout, np.float32)

    assign_all = np.stack(
        [(X[b] @ Wc.T + bc).argmax(-1) for b in range(B)]
    )
    T2, Ws, As = make_schedule(assign_all)
    order = np.argsort(
        -np.stack([np.bincount(a, minlength=C) for a in assign_all]),
        axis=1, kind="stable")

    per_batch = []
    poss = []
    for b in range(B):
        a = assign_all[b]
        X2 = np.zeros((T2, D), np.float32)
        pos = np.empty(T, np.int64)
        patch = np.empty((C, LH, 64), np.float32)
        for s in range(C):
            c = order[b, s]
            toks = np.nonzero(a == c)[0]
            n = len(toks)
            A = As[s]
            pad_cnt = Ws[s] - 1 - n
            patch[s] = float(T - n - pad_cnt)
            X2[A + 1:A + 1 + n] = X[b, toks]
            pos[toks] = np.arange(A + 1, A + 1 + n)
        per_batch.append(
            {
                "xt": np.ascontiguousarray(X2.T),
                "patch": patch[None].astype(np.float16),
            }
        )
        poss.append(pos)

    per_half = []
    for hh in range(2):
        r = slice(hh * 256, (hh + 1) * 256)
        wqk = np.concatenate([Win[0:D][r].T, Win[D:2 * D][r].T], axis=1)
        per_half.append(
            {
                "wqk": np.ascontiguousarray(wqk),
                "wv": np.ascontiguousarray(Win[2 * D:][r].T),
                "wo": np.ascontiguousarray(Wout[:, r].T),
            }
        )

    in_maps = [dict(per_batch[g // 2], **per_half[g % 2]) for g in range(8)]
    return (T2, tuple(Ws), tuple(As)), in_maps, poss


_NC_CACHE = {}


def kernel(X, Wc, bc, Win, bin_, Wout, bout):
    assert not np.any(np.asarray(bin_)), "kernel assumes zero in_proj bias"
    sched, in_maps, poss = prepare(X, Wc, bc, Win, Wout)
    if sched not in _NC_CACHE:
        _NC_CACHE[sched] = build_nc(sched[0], list(sched[1]), list(sched[2]))
    nc = _NC_CACHE[sched]
    res = run_bass_kernel_spmd(nc, in_maps, core_ids=list(range(8)))
    outs = res.results
    bout = np.asarray(bout, np.float32)
    out = np.empty((B, T, D), np.float32)
    for b in range(B):
        full = outs[2 * b]["outT"].astype(np.float32) + \
            outs[2 * b + 1]["outT"].astype(np.float32)
        out[b] = full.T[poss[b]] + bout
    return out
